# revision 62
# baseline (speedup 1.0000x reference)
"""Multi-head self-attention on 8 Trainium2 NeuronCores (~238 us HW).

Tensor-parallel over heads: core c owns heads 2c, 2c+1 (128 of the 1024
hidden columns).  The host pre-transposes x to x^T [1024, 4096] bf16 so
every stage flows through the PE with zero on-chip layout fixups:
  1. Q^T/K^T = (w.T @ x^T + b) in [d, token] layout (2 heads stacked on
     partitions: 0:64 head0, 64:128 head1).
  2. V^T likewise (per-partition bias via tensor_scalar_add), then PE
     transposes into V_aug [token, 65-per-head] where column 64/129 = 1.0
     (the ones column makes the softmax denominator fall out of P@V).
  3. Attention in 4 chunks of (batch, 1024 queries), software-pipelined:
     scores^T tiles = K^T.T @ Q^T (K=64 contraction; the two heads'
     matmuls run concurrently in disjoint PE row groups), P^T =
     exp(S^T/8) on ScalarE (no max subtraction: |S/8| < 3, exp can't
     overflow), and the previous chunk's P@V accumulation + the
     projection/WO back-work are interleaved so the PE fills the gaps
     while ScalarE (the bottleneck engine, ~1.1us per [128,1024] exp)
     streams.  out^T[65, s] = V_aug.T @ P^T; row 64 is the softmax
     denominator; normalize = reciprocal_approx_fast + gpsimd
     partition_broadcast + one multiply.
  4. partial = attnT.T @ wo[128 rows of this core] -> HBM (bf16)
Host sums the 8 partials and adds bo.

Scheduling notes (hard-won): ScalarE must never starve, so scores for
step tt+1 are emitted before the slower PE work of step tt; PSUM is
exactly 8 banks = scores pair (4) + two P@V half-accumulators (2) +
extras pool (2); the PT pool (exp outputs) needs ~46 tiles because the
previous chunk's tiles are only released during the second P@V half.
PE is HAM-throttled to 1.2 GHz unless kept busy, hence the identity
warm-up matmuls during the initial DMA window.

Shapes hardcoded for x:[2,2048,1024], 16 heads, d_k=64.
"""

import numpy as np
import ml_dtypes

import concourse.bass as bass
import concourse.tile as tile
from concourse import bacc, mybir
from concourse.bass import ts
from concourse.bass_utils import run_bass_kernel_spmd

BF16 = mybir.dt.bfloat16
FP8 = mybir.dt.float8e4
F32 = mybir.dt.float32
NPBF16 = ml_dtypes.bfloat16

B = 2
S = 2048
D = 1024
NT = B * S  # 4096 tokens
DK = 64
NCORES = 8
HPC = 2  # heads per core
SC = 1024  # attention s-chunk (exp op free size)

_CACHE = {}


def _build_nc():
    nc = bacc.Bacc("TRN2", target_bir_lowering=False, debug=False,
                   num_devices=NCORES)

    xT = nc.dram_tensor("xT", [D, NT], BF16, kind="ExternalInput").ap()
    wq = nc.dram_tensor("wq", [D, 128], BF16, kind="ExternalInput").ap()
    wk = nc.dram_tensor("wk", [D, 128], BF16, kind="ExternalInput").ap()
    wv = nc.dram_tensor("wv", [D, 128], BF16, kind="ExternalInput").ap()
    bq = nc.dram_tensor("bq", [128, 1], F32, kind="ExternalInput").ap()
    bk = nc.dram_tensor("bk", [128, 1], F32, kind="ExternalInput").ap()
    bv = nc.dram_tensor("bv", [128, 1], F32, kind="ExternalInput").ap()
    wo = nc.dram_tensor("wo", [128, D], BF16, kind="ExternalInput").ap()
    out = nc.dram_tensor("out", [NT, D], BF16, kind="ExternalOutput").ap()

    with tile.TileContext(nc) as tc:
        _emit(nc, tc, xT, wq, wk, wv, bq, bk, bv, wo, out)
    nc.compile()
    return nc


def _emit(nc, tc, xT, wq, wk, wv, bq, bk, bv, wo, out):
    import contextlib
    ctx = contextlib.ExitStack()
    with ctx:
        consts = ctx.enter_context(tc.tile_pool(name="consts", bufs=1))
        ptp = ctx.enter_context(tc.tile_pool(name="ptp", bufs=47))
        psp = ctx.enter_context(tc.tile_pool(name="psp", bufs=2, space="PSUM"))
        pvp = ctx.enter_context(tc.tile_pool(name="pvp", bufs=2, space="PSUM"))
        pse = ctx.enter_context(tc.tile_pool(name="pse", bufs=2, space="PSUM"))
        stg = ctx.enter_context(tc.tile_pool(name="stg", bufs=2))
        nrm = ctx.enter_context(tc.tile_pool(name="nrm", bufs=1))

        # ---- persistent SBUF tensors ----
        xT_sb = consts.tile([128, 8, NT], BF16)      # 8 k-tiles of x^T
        wq_sb = consts.tile([128, 8, 128], BF16)
        wk_sb = consts.tile([128, 8, 128], BF16)
        wv_sb = consts.tile([128, 8, 128], BF16)
        bq_sb = consts.tile([128, 1], F32)
        bk_sb = consts.tile([128, 1], F32)
        bv_sb = consts.tile([128, 1], F32)
        wo_sb = consts.tile([128, D], BF16)
        QT = consts.tile([128, NT], BF16)
        KT = consts.tile([128, NT], BF16)
        V_sb = consts.tile([128, 32, 130], BF16)     # [t-in-tile, t_tile, col]
        attnT = consts.tile([128, NT], BF16)
        ident = consts.tile([128, 128], BF16)
        vstg = ctx.enter_context(tc.tile_pool(name="vstg", bufs=2))

        xT_d = xT.rearrange("(k p) n -> k p n", p=128)
        # x^T lands in first-use order, alternating the sync/scalar DMA
        # queues: the first 512 columns of every k-tile arrive within a few
        # us so the prologue projections are never DMA-stalled.
        def xdma(k, lo, hi):
            eng = nc.sync if k % 2 == 0 else nc.scalar
            eng.dma_start(out=xT_sb[:, k, lo:hi], in_=xT_d[k][:, lo:hi])
        for k in range(8):
            xdma(k, 0, 512)
        for k in range(8):
            xdma(k, 512, 1024)
        # identity (gpsimd) first so the PE warm-up can start immediately
        from concourse.masks import make_identity
        make_identity(nc, ident)
        wups = pse.tile([128, 128], F32, tag="pse", name="wups")
        for i in range(40):
            nc.tensor.matmul(wups, lhsT=ident, rhs=ident, start=True, stop=True)
        nc.gpsimd.dma_start(out=bq_sb, in_=bq)
        nc.gpsimd.dma_start(out=bk_sb, in_=bk)
        nc.gpsimd.dma_start(out=bv_sb, in_=bv)
        nc.gpsimd.dma_start(out=wk_sb, in_=wk.rearrange("(k p) c -> p k c", p=128))
        nc.gpsimd.dma_start(out=wq_sb, in_=wq.rearrange("(k p) c -> p k c", p=128))
        nc.gpsimd.dma_start(out=wv_sb, in_=wv.rearrange("(k p) c -> p k c", p=128))
        nc.gpsimd.dma_start(out=wo_sb, in_=wo)
        for k in range(8):
            xdma(k, 1024, 2048)

        # trigger the exp ACT-table load early (~2.7us) while DMAs run
        tblw = stg.tile([128, 1], F32, tag="ob")
        nc.scalar.activation(out=tblw, in_=bq_sb,
                             func=mybir.ActivationFunctionType.Exp)

        # ones columns of V_aug (never touched by the per-tile copies)
        nc.vector.memset(V_sb[:, :, 64:65], 1.0)
        nc.vector.memset(V_sb[:, :, 129:130], 1.0)
        for k in range(8):
            xdma(k, S, NT)

        # ---- emit helpers (psum shared with the scores tag) ----
        vt_stage = {}

        def emit_v_proj(c):
            # V^T chunk: [c128, 512 tokens] += wv[k].T @ xT[k] (+bias, ->bf16)
            psv = pse.tile([128, 512], F32, tag="pse")
            for k in range(8):
                nc.tensor.matmul(psv, lhsT=wv_sb[:, k, :],
                                 rhs=xT_sb[:, k, ts(c, 512)],
                                 start=(k == 0), stop=(k == 7))
            vt = vstg.tile([128, 512], BF16, tag="vt", name=f"vt{c}")
            nc.vector.tensor_scalar_add(vt, psv, bv_sb)
            vt_stage[c] = vt

        def emit_v_tr(tt):
            # transpose one 128x128 block of V^T into V_aug [t, col] layout
            c, j = divmod(tt, 4)
            trp = pse.tile([128, 128], BF16, tag="pse", name=f"trp{tt}")
            nc.tensor.transpose(trp, vt_stage[c][:, ts(j, 128)], ident)
            nc.vector.tensor_copy(V_sb[:, tt, 0:64], trp[:, 0:64])
            nc.vector.tensor_copy(V_sb[:, tt, 65:129], trp[:, 64:128])

        def emit_wo_tile(tt, use_act=False):
            for eh in range(2):
                pw = pse.tile([128, 512], F32, tag="pse")
                nc.tensor.matmul(pw, lhsT=attnT[:, ts(tt, 128)],
                                 rhs=wo_sb[:, ts(eh, 512)],
                                 start=True, stop=True)
                ob = stg.tile([128, 512], BF16, tag="ob")
                if use_act and eh == 1:
                    nc.scalar.activation(
                        out=ob, in_=pw,
                        func=mybir.ActivationFunctionType.Copy, bias=0.0)
                else:
                    nc.vector.tensor_copy(ob, pw)
                nc.sync.dma_start(
                    out=out[tt * 128:(tt + 1) * 128, eh * 512:(eh + 1) * 512],
                    in_=ob)

        def emit_proj_chunk(w_sb, b_sb, o_sb, n, w=512):
            # w-token chunk n (units of w) of the Q^T or K^T projection
            ps = pse.tile([128, 512], F32, tag="pse")
            for k in range(8):
                nc.tensor.matmul(ps[:, 0:w], lhsT=w_sb[:, k, :],
                                 rhs=xT_sb[:, k, ts(n, w)],
                                 start=(k == 0), stop=(k == 7))
            nc.vector.tensor_scalar_add(o_sb[:, ts(n, w)], ps[:, 0:w], b_sb)

        def emit_normalize_half(prev, c):
            b, sc, pts, pv_state = prev
            s0 = b * S + sc * SC + c * 512
            for h in range(HPC):
                pso = pv_state['psos'][h]
                # stage the accumulator out of PSUM first so the bank frees
                # for the next PV half after two copies, not the whole chain.
                # bf16 staging is free precision-wise: attnT is bf16 anyway,
                # and rounding the numerator before the divide is equivalent.
                rsum = nrm.tile([1, 512], F32, tag="rsum")
                nc.vector.tensor_copy(rsum, pso[64:65, :])
                ostg = nrm.tile([64, 512], BF16, tag="ostg", name=f"ostg{h}")
                nc.vector.tensor_copy(ostg, pso[0:64, :])
                nc.vector.reciprocal_approx_fast(out=rsum, in_=rsum)
                recb = nrm.tile([64, 512], F32, tag="recb")
                nc.gpsimd.partition_broadcast(recb, rsum)
                nc.vector.tensor_mul(
                    attnT[h * DK:(h + 1) * DK, s0:s0 + 512],
                    ostg, recb)

        def emit_pv_step(prev, s):
            # one pipeline step of P@V for the previous chunk: two t-tiles
            # into the [65, 512] accumulators of half-chunk c = s // 8
            b, sc, pts, pv_state = prev
            c = s // 8
            if s % 8 == 0:
                pv_state['psos'] = [
                    pvp.tile([128, 512], F32, tag="pv",
                             name=f"pso{b}_{sc}_{c}_{h}")
                    for h in range(HPC)]
            psos = pv_state['psos']
            for dt in range(2):
                tt = 2 * (s % 8) + dt
                for h in range(HPC):
                    nc.tensor.matmul(
                        psos[h][0:65, :],
                        lhsT=V_sb[:, b * 16 + tt, h * 65:(h + 1) * 65],
                        rhs=pts[tt][h][:, ts(c, 512)],
                        start=(tt == 0), stop=(tt == 15))
            if s % 8 == 7:
                emit_normalize_half(prev, c)

        # ---- minimal prologue: just enough of Q^T/K^T for chunk 0.
        # KT0 and QT0 accumulate interleaved at k granularity so both chase
        # each arriving x^T k-tile DMA instead of serializing.
        psK = pse.tile([128, 512], F32, tag="pse", name="psK")
        psQ = pse.tile([128, 512], F32, tag="pse", name="psQ")
        for k in range(8):
            nc.tensor.matmul(psK, lhsT=wk_sb[:, k, :],
                             rhs=xT_sb[:, k, 0:512],
                             start=(k == 0), stop=(k == 7))
            nc.tensor.matmul(psQ, lhsT=wq_sb[:, k, :],
                             rhs=xT_sb[:, k, 0:512],
                             start=(k == 0), stop=(k == 7))
        nc.vector.tensor_scalar_add(KT[:, 0:512], psK, bk_sb)
        nc.vector.tensor_scalar_add(QT[:, 0:512], psQ, bq_sb)
        emit_proj_chunk(wq_sb, bq_sb, QT, 1)

        # deferred PE work, interleaved into the ACT-bound attention loop.
        # entry = (step, thunk): emitted at the given tt step of that chunk.
        # Q^T/K^T chunks are emitted at 256-wide granularity (n in units of
        # 256) so no single extra hogs the PE long enough to starve ScalarE.
        qk = [(wq_sb, bq_sb, QT), (wk_sb, bk_sb, KT)]

        def pj(which, n256):
            return lambda: emit_proj_chunk(*qk[which], n256, w=256)

        def spread(thunks, start, stop):
            # distribute thunks evenly over tt steps [start, stop)
            n = len(thunks)
            return [(start + (i * (stop - start)) // n, t)
                    for i, t in enumerate(thunks)]

        def v_extras(c0):
            # V^T proj chunk at step 4c, its 4 transposes right after
            ex = []
            for c in range(c0, c0 + 4):
                st = (c - c0) * 4
                ex.append((st, lambda c=c: emit_v_proj(c)))
                ex += [(st + 1 + j, lambda t=4 * c + j: emit_v_tr(t))
                       for j in range(4)]
            return ex

        extras_per_chunk = [
            # chunk 0 (b0,sc0): KT 1-3 just ahead of use, QT 2-3 (for sc1),
            # V tiles 0-15 (b0, needed by chunk 1)
            spread([pj(1, n) for n in (2, 3)], 0, 3)
            + spread([pj(1, n) for n in (4, 5, 6, 7)], 3, 11)
            + spread([pj(0, n) for n in (4, 5, 6, 7)], 8, 16)
            + v_extras(0),
            # chunk 1 (b0,sc1): QT 4-5, KT 4, V 16-31 (b1)
            spread([pj(0, n) for n in (8, 9, 10, 11)], 0, 8)
            + spread([pj(1, n) for n in (8, 9)], 8, 12)
            + v_extras(4),
            # chunk 2 (b1,sc0): KT 5-7 ahead of use, QT 6-7, WO tiles 0-7
            spread([pj(1, n) for n in (10, 11, 12, 13)], 0, 8)
            + spread([pj(1, n) for n in (14, 15)], 8, 12)
            + spread([pj(0, n) for n in (12, 13, 14, 15)], 4, 12)
            + [(2 * i, lambda t=t: emit_wo_tile(t)) for i, t in enumerate(range(8))],
            # chunk 3 (b1,sc1): WO tiles 8-15 (odd steps: chunk starts are
            # already congested by the PV/normalize handoff)
            [(2 * i + 1, lambda t=t: emit_wo_tile(t)) for i, t in enumerate(range(8, 16))],
        ]

        def emit_scores(b, sc, tt):
            # one t-tile of S^T for both heads -> psum pair; returns the pair
            s0 = b * S + sc * SC
            pair = []
            for h in range(HPC):
                ps = psp.tile([128, SC], F32, tag="ps")
                hsl = slice(h * DK, (h + 1) * DK)
                for n2 in range(SC // 512):
                    nc.tensor.matmul(
                        ps[:, ts(n2, 512)],
                        lhsT=KT[hsl, b * S + tt * 128:b * S + (tt + 1) * 128],
                        rhs=QT[hsl, s0 + n2 * 512:s0 + (n2 + 1) * 512],
                        start=True, stop=True)
                pair.append(ps)
            return pair

        chunks = [(b, sc) for b in range(B) for sc in range(S // SC)]
        prev = None
        for ci, (b, sc) in enumerate(chunks):
            extras = sorted(extras_per_chunk[ci], key=lambda e: e[0])
            pts = []
            cur = (b, sc, pts, {})
            pair = emit_scores(b, sc, 0)
            for tt in range(16):
                row = []
                for h in range(HPC):
                    pt = ptp.tile([128, SC], BF16, tag="pt")
                    nc.scalar.activation(
                        out=pt, in_=pair[h],
                        func=mybir.ActivationFunctionType.Exp,
                        scale=0.125)
                    row.append(pt)
                pts.append(row)
                # emit next scores ahead of the slower PE work so ScalarE's
                # psum slots refill as soon as its exp frees them
                if tt + 1 < 16:
                    pair = emit_scores(b, sc, tt + 1)
                if prev is not None:
                    emit_pv_step(prev, tt)
                while extras and extras[0][0] <= tt:
                    extras.pop(0)[1]()
            for _, e in extras:
                e()
            prev = cur
        # tail: PV + normalize for the last chunk, with WO 16-23 (already
        # normalized) interleaved; then the final WO tiles
        for s in range(16):
            emit_pv_step(prev, s)
            if s % 2 == 1:
                emit_wo_tile(16 + s // 2, use_act=True)
        for tt in range(24, 32):
            emit_wo_tile(tt, use_act=True)


def _prep_in_maps(x, wq, bq, wk, bk, wv, bv, wo):
    x2 = np.asarray(x, np.float32).reshape(NT, D)
    xT = np.ascontiguousarray(x2.T).astype(NPBF16)
    wq = np.asarray(wq, np.float32)
    wk = np.asarray(wk, np.float32)
    wv = np.asarray(wv, np.float32)
    wo = np.asarray(wo, np.float32)
    bq = np.asarray(bq, np.float32)
    bk = np.asarray(bk, np.float32)
    bv = np.asarray(bv, np.float32)
    in_maps = []
    for c in range(NCORES):
        cs = slice(c * 128, (c + 1) * 128)
        in_maps.append({
            "xT": xT,
            "wq": wq[:, cs].astype(NPBF16),
            "wk": wk[:, cs].astype(NPBF16),
            "wv": wv[:, cs].astype(NPBF16),
            "bq": np.ascontiguousarray(bq[cs].reshape(128, 1)),
            "bk": np.ascontiguousarray(bk[cs].reshape(128, 1)),
            "bv": np.ascontiguousarray(bv[cs].reshape(128, 1)),
            "wo": wo[cs, :].astype(NPBF16),
        })
    return in_maps


def kernel(x, wq, bq, wk, bk, wv, bv, wo, bo, _run_kwargs=None):
    if "nc" not in _CACHE:
        _CACHE["nc"] = _build_nc()
    nc = _CACHE["nc"]
    in_maps = _prep_in_maps(x, wq, bq, wk, bk, wv, bv, wo)
    res = run_bass_kernel_spmd(nc, in_maps, list(range(NCORES)),
                               **(_run_kwargs or {}))
    acc = np.zeros((NT, D), np.float32)
    for c in range(NCORES):
        acc += res.results[c]["out"].astype(np.float32)
    acc += np.asarray(bo, np.float32)[None, :]
    if _run_kwargs:
        _CACHE["last_results"] = res
    return acc.reshape(B, S, D)


# revision 63
# speedup vs baseline: 1.0647x; 1.0647x over previous
"""Multi-head self-attention on 8 Trainium2 NeuronCores (~238 us HW).

Tensor-parallel over heads: core c owns heads 2c, 2c+1 (128 of the 1024
hidden columns).  The host pre-transposes x to x^T [1024, 4096] bf16 so
every stage flows through the PE with zero on-chip layout fixups:
  1. Q^T/K^T = (w.T @ x^T + b) in [d, token] layout (2 heads stacked on
     partitions: 0:64 head0, 64:128 head1).
  2. V^T likewise (per-partition bias via tensor_scalar_add), then PE
     transposes into V_aug [token, 65-per-head] where column 64/129 = 1.0
     (the ones column makes the softmax denominator fall out of P@V).
  3. Attention in 4 chunks of (batch, 1024 queries), software-pipelined:
     scores^T tiles = K^T.T @ Q^T (K=64 contraction; the two heads'
     matmuls run concurrently in disjoint PE row groups), P^T =
     exp(S^T/8) on ScalarE (no max subtraction: |S/8| < 3, exp can't
     overflow), and the previous chunk's P@V accumulation + the
     projection/WO back-work are interleaved so the PE fills the gaps
     while ScalarE (the bottleneck engine, ~1.1us per [128,1024] exp)
     streams.  out^T[65, s] = V_aug.T @ P^T; row 64 is the softmax
     denominator; normalize = reciprocal_approx_fast + gpsimd
     partition_broadcast + one multiply.
  4. partial = attnT.T @ wo[128 rows of this core] -> HBM (bf16)
Host sums the 8 partials and adds bo.

Scheduling notes (hard-won): ScalarE must never starve, so scores for
step tt+1 are emitted before the slower PE work of step tt; PSUM is
exactly 8 banks = scores pair (4) + two P@V half-accumulators (2) +
extras pool (2); the PT pool (exp outputs) needs ~46 tiles because the
previous chunk's tiles are only released during the second P@V half.
PE is HAM-throttled to 1.2 GHz unless kept busy, hence the identity
warm-up matmuls during the initial DMA window.

Shapes hardcoded for x:[2,2048,1024], 16 heads, d_k=64.
"""

import numpy as np
import ml_dtypes

import concourse.bass as bass
import concourse.tile as tile
from concourse import bacc, mybir
from concourse.bass import ts
from concourse.bass_utils import run_bass_kernel_spmd

BF16 = mybir.dt.bfloat16
FP8 = mybir.dt.float8e4
F32 = mybir.dt.float32
NPBF16 = ml_dtypes.bfloat16

B = 2
S = 2048
D = 1024
NT = B * S  # 4096 tokens
DK = 64
NCORES = 8
HPC = 2  # heads per core
SC = 1024  # attention s-chunk (exp op free size)

_CACHE = {}


def _build_nc():
    nc = bacc.Bacc("TRN2", target_bir_lowering=False, debug=False,
                   num_devices=NCORES)

    xT = nc.dram_tensor("xT", [D, NT], BF16, kind="ExternalInput").ap()
    wq = nc.dram_tensor("wq", [D, 128], BF16, kind="ExternalInput").ap()
    wk = nc.dram_tensor("wk", [D, 128], BF16, kind="ExternalInput").ap()
    wv = nc.dram_tensor("wv", [D, 128], BF16, kind="ExternalInput").ap()
    bq = nc.dram_tensor("bq", [128, 1], F32, kind="ExternalInput").ap()
    bk = nc.dram_tensor("bk", [128, 1], F32, kind="ExternalInput").ap()
    bv = nc.dram_tensor("bv", [128, 1], F32, kind="ExternalInput").ap()
    wo = nc.dram_tensor("wo", [128, D], BF16, kind="ExternalInput").ap()
    out = nc.dram_tensor("out", [NT, D], BF16, kind="ExternalOutput").ap()

    with tile.TileContext(nc) as tc:
        _emit(nc, tc, xT, wq, wk, wv, bq, bk, bv, wo, out)
    nc.compile()
    return nc


def _emit(nc, tc, xT, wq, wk, wv, bq, bk, bv, wo, out):
    import contextlib
    ctx = contextlib.ExitStack()
    with ctx:
        consts = ctx.enter_context(tc.tile_pool(name="consts", bufs=1))
        ptp = ctx.enter_context(tc.tile_pool(name="ptp", bufs=46))
        psp = ctx.enter_context(tc.tile_pool(name="psp", bufs=2, space="PSUM"))
        pvp = ctx.enter_context(tc.tile_pool(name="pvp", bufs=2, space="PSUM"))
        pse = ctx.enter_context(tc.tile_pool(name="pse", bufs=2, space="PSUM"))
        stg = ctx.enter_context(tc.tile_pool(name="stg", bufs=3))
        nrm = ctx.enter_context(tc.tile_pool(name="nrm", bufs=1))

        # ---- persistent SBUF tensors ----
        xT_sb = consts.tile([128, 8, NT], BF16)      # 8 k-tiles of x^T
        wq_sb = consts.tile([128, 8, 128], BF16)
        wk_sb = consts.tile([128, 8, 128], BF16)
        wv_sb = consts.tile([128, 8, 128], BF16)
        bq_sb = consts.tile([128, 1], F32)
        bk_sb = consts.tile([128, 1], F32)
        bv_sb = consts.tile([128, 1], F32)
        wo_sb = consts.tile([128, D], BF16)
        QT = consts.tile([128, NT], BF16)
        KT = consts.tile([128, NT], BF16)
        V_sb = consts.tile([128, 32, 130], BF16)     # [t-in-tile, t_tile, col]
        attnT = consts.tile([128, NT], BF16)
        ident = consts.tile([128, 128], BF16)
        vstg = ctx.enter_context(tc.tile_pool(name="vstg", bufs=2))

        xT_d = xT.rearrange("(k p) n -> k p n", p=128)
        # x^T lands in first-use order, alternating the sync/scalar DMA
        # queues: the first 512 columns of every k-tile arrive within a few
        # us so the prologue projections are never DMA-stalled.
        def xdma(k, lo, hi):
            eng = nc.sync if k % 2 == 0 else nc.scalar
            eng.dma_start(out=xT_sb[:, k, lo:hi], in_=xT_d[k][:, lo:hi])
        for k in range(8):
            xdma(k, 0, 512)
        for k in range(8):
            xdma(k, 512, 1024)
        # identity (gpsimd) first so the PE warm-up can start immediately
        from concourse.masks import make_identity
        make_identity(nc, ident)
        wups = pse.tile([128, 128], F32, tag="pse", name="wups")
        for i in range(40):
            nc.tensor.matmul(wups, lhsT=ident, rhs=ident, start=True, stop=True)
        nc.gpsimd.dma_start(out=bq_sb, in_=bq)
        nc.gpsimd.dma_start(out=bk_sb, in_=bk)
        nc.gpsimd.dma_start(out=bv_sb, in_=bv)
        nc.gpsimd.dma_start(out=wk_sb, in_=wk.rearrange("(k p) c -> p k c", p=128))
        nc.gpsimd.dma_start(out=wq_sb, in_=wq.rearrange("(k p) c -> p k c", p=128))
        nc.gpsimd.dma_start(out=wv_sb, in_=wv.rearrange("(k p) c -> p k c", p=128))
        nc.gpsimd.dma_start(out=wo_sb, in_=wo)
        for k in range(8):
            xdma(k, 1024, 2048)

        # trigger the exp ACT-table load early (~2.7us) while DMAs run
        tblw = stg.tile([128, 1], F32, tag="ob")
        nc.scalar.activation(out=tblw, in_=bq_sb,
                             func=mybir.ActivationFunctionType.Exp)

        # ones columns of V_aug (never touched by the per-tile copies)
        nc.vector.memset(V_sb[:, :, 64:65], 1.0)
        nc.vector.memset(V_sb[:, :, 129:130], 1.0)
        for k in range(8):
            xdma(k, S, NT)

        # ---- emit helpers (psum shared with the scores tag) ----
        vt_stage = {}

        def emit_v_proj(c):
            # V^T chunk: [c128, 512 tokens] += wv[k].T @ xT[k] (+bias, ->bf16)
            psv = pse.tile([128, 512], F32, tag="pse")
            for k in range(8):
                nc.tensor.matmul(psv, lhsT=wv_sb[:, k, :],
                                 rhs=xT_sb[:, k, ts(c, 512)],
                                 start=(k == 0), stop=(k == 7))
            vt = vstg.tile([128, 512], BF16, tag="vt", name=f"vt{c}")
            nc.vector.tensor_scalar_add(vt, psv, bv_sb)
            vt_stage[c] = vt

        def emit_v_tr(tt):
            # transpose one 128x128 block of V^T into V_aug [t, col] layout
            c, j = divmod(tt, 4)
            trp = pse.tile([128, 128], BF16, tag="pse", name=f"trp{tt}")
            nc.tensor.transpose(trp, vt_stage[c][:, ts(j, 128)], ident)
            nc.vector.tensor_copy(V_sb[:, tt, 0:64], trp[:, 0:64])
            nc.vector.tensor_copy(V_sb[:, tt, 65:129], trp[:, 64:128])

        def emit_wo_tile(tt, use_act=False):
            for eh in range(2):
                pw = pse.tile([128, 512], F32, tag="pse")
                nc.tensor.matmul(pw, lhsT=attnT[:, ts(tt, 128)],
                                 rhs=wo_sb[:, ts(eh, 512)],
                                 start=True, stop=True)
                ob = stg.tile([128, 512], BF16, tag="ob")
                if use_act and eh == 1:
                    nc.scalar.activation(
                        out=ob, in_=pw,
                        func=mybir.ActivationFunctionType.Copy, bias=0.0)
                else:
                    nc.vector.tensor_copy(ob, pw)
                nc.sync.dma_start(
                    out=out[tt * 128:(tt + 1) * 128, eh * 512:(eh + 1) * 512],
                    in_=ob)

        def emit_proj_chunk(w_sb, b_sb, o_sb, n, w=512):
            # w-token chunk n (units of w) of the Q^T or K^T projection
            ps = pse.tile([128, 512], F32, tag="pse")
            for k in range(8):
                nc.tensor.matmul(ps[:, 0:w], lhsT=w_sb[:, k, :],
                                 rhs=xT_sb[:, k, ts(n, w)],
                                 start=(k == 0), stop=(k == 7))
            nc.vector.tensor_scalar_add(o_sb[:, ts(n, w)], ps[:, 0:w], b_sb)

        def emit_normalize_half(prev, c):
            b, sc, pts, pv_state = prev
            s0 = b * S + sc * SC + c * 512
            for h in range(HPC):
                pso = pv_state['psos'][h]
                # stage the accumulator out of PSUM first so the bank frees
                # for the next PV half after one copy, not the whole chain
                ostg = nrm.tile([65, 512], F32, tag="ostg", name=f"ostg{h}")
                nc.vector.tensor_copy(ostg, pso[0:65, :])
                rsum = nrm.tile([1, 512], F32, tag="rsum")
                nc.vector.tensor_copy(rsum, ostg[64:65, :])
                nc.vector.reciprocal_approx_fast(out=rsum, in_=rsum)
                recb = nrm.tile([64, 512], F32, tag="recb")
                nc.gpsimd.partition_broadcast(recb, rsum)
                nc.vector.tensor_mul(
                    attnT[h * DK:(h + 1) * DK, s0:s0 + 512],
                    ostg[0:64, :], recb)

        def emit_pv_step(prev, s):
            # one pipeline step of P@V for the previous chunk: two t-tiles
            # into the [65, 512] accumulators of half-chunk c = s // 8
            b, sc, pts, pv_state = prev
            c = s // 8
            if s % 8 == 0:
                pv_state['psos'] = [
                    pvp.tile([128, 512], F32, tag="pv",
                             name=f"pso{b}_{sc}_{c}_{h}")
                    for h in range(HPC)]
            psos = pv_state['psos']
            for dt in range(2):
                tt = 2 * (s % 8) + dt
                for h in range(HPC):
                    nc.tensor.matmul(
                        psos[h][0:65, :],
                        lhsT=V_sb[:, b * 16 + tt, h * 65:(h + 1) * 65],
                        rhs=pts[tt][h][:, ts(c, 512)],
                        start=(tt == 0), stop=(tt == 15))
            if s % 8 == 7:
                emit_normalize_half(prev, c)

        # ---- minimal prologue: just enough of Q^T/K^T for chunk 0.
        # KT0 and QT0 accumulate interleaved at k granularity so both chase
        # each arriving x^T k-tile DMA instead of serializing.
        psK = pse.tile([128, 512], F32, tag="pse", name="psK")
        psQ = pse.tile([128, 512], F32, tag="pse", name="psQ")
        for k in range(8):
            nc.tensor.matmul(psK, lhsT=wk_sb[:, k, :],
                             rhs=xT_sb[:, k, 0:512],
                             start=(k == 0), stop=(k == 7))
            nc.tensor.matmul(psQ, lhsT=wq_sb[:, k, :],
                             rhs=xT_sb[:, k, 0:512],
                             start=(k == 0), stop=(k == 7))
        nc.vector.tensor_scalar_add(KT[:, 0:512], psK, bk_sb)
        nc.vector.tensor_scalar_add(QT[:, 0:512], psQ, bq_sb)
        emit_proj_chunk(wq_sb, bq_sb, QT, 1)

        # deferred PE work, interleaved into the ACT-bound attention loop.
        # entry = (step, thunk): emitted at the given tt step of that chunk.
        # Q^T/K^T chunks are emitted at 256-wide granularity (n in units of
        # 256) so no single extra hogs the PE long enough to starve ScalarE.
        qk = [(wq_sb, bq_sb, QT), (wk_sb, bk_sb, KT)]

        def pj(which, n256):
            return lambda: emit_proj_chunk(*qk[which], n256, w=256)

        def spread(thunks, start, stop):
            # distribute thunks evenly over tt steps [start, stop)
            n = len(thunks)
            return [(start + (i * (stop - start)) // n, t)
                    for i, t in enumerate(thunks)]

        def v_extras(c0):
            # V^T proj chunk at step 4c, its 4 transposes right after
            ex = []
            for c in range(c0, c0 + 4):
                st = (c - c0) * 4
                ex.append((st, lambda c=c: emit_v_proj(c)))
                ex += [(st + 1 + j, lambda t=4 * c + j: emit_v_tr(t))
                       for j in range(4)]
            return ex

        extras_per_chunk = [
            # chunk 0 (b0,sc0): KT 1-3 just ahead of use, QT 2-3 (for sc1),
            # V tiles 0-15 (b0, needed by chunk 1)
            spread([pj(1, n) for n in (2, 3)], 0, 3)
            + spread([pj(1, n) for n in (4, 5, 6, 7)], 3, 11)
            + spread([pj(0, n) for n in (4, 5, 6, 7)], 8, 16)
            + v_extras(0),
            # chunk 1 (b0,sc1): QT 4-5, KT 4, V 16-31 (b1)
            spread([pj(0, n) for n in (8, 9, 10, 11)], 0, 8)
            + spread([pj(1, n) for n in (8, 9)], 8, 12)
            + v_extras(4),
            # chunk 2 (b1,sc0): KT 5-7 ahead of use, QT 6-7, WO tiles 0-7
            spread([pj(1, n) for n in (10, 11, 12, 13)], 0, 8)
            + spread([pj(1, n) for n in (14, 15)], 8, 12)
            + spread([pj(0, n) for n in (12, 13, 14, 15)], 4, 12)
            + [(2 * i, lambda t=t: emit_wo_tile(t)) for i, t in enumerate(range(8))],
            # chunk 3 (b1,sc1): WO tiles 8-15 (odd steps: chunk starts are
            # already congested by the PV/normalize handoff)
            [(2 * i + 1, lambda t=t: emit_wo_tile(t)) for i, t in enumerate(range(8, 16))],
        ]

        def emit_scores(b, sc, tt):
            # one t-tile of S^T for both heads -> psum pair; returns the pair
            s0 = b * S + sc * SC
            pair = []
            for h in range(HPC):
                ps = psp.tile([128, SC], F32, tag="ps")
                hsl = slice(h * DK, (h + 1) * DK)
                for n2 in range(SC // 512):
                    nc.tensor.matmul(
                        ps[:, ts(n2, 512)],
                        lhsT=KT[hsl, b * S + tt * 128:b * S + (tt + 1) * 128],
                        rhs=QT[hsl, s0 + n2 * 512:s0 + (n2 + 1) * 512],
                        start=True, stop=True)
                pair.append(ps)
            return pair

        chunks = [(b, sc) for b in range(B) for sc in range(S // SC)]
        prev = None
        for ci, (b, sc) in enumerate(chunks):
            extras = sorted(extras_per_chunk[ci], key=lambda e: e[0])
            pts = []
            cur = (b, sc, pts, {})
            pair = emit_scores(b, sc, 0)
            for tt in range(16):
                row = []
                for h in range(HPC):
                    pt = ptp.tile([128, SC], BF16, tag="pt")
                    nc.scalar.activation(
                        out=pt, in_=pair[h],
                        func=mybir.ActivationFunctionType.Exp,
                        scale=0.125)
                    row.append(pt)
                pts.append(row)
                # emit next scores ahead of the slower PE work so ScalarE's
                # psum slots refill as soon as its exp frees them
                if tt + 1 < 16:
                    pair = emit_scores(b, sc, tt + 1)
                if prev is not None:
                    emit_pv_step(prev, tt)
                while extras and extras[0][0] <= tt:
                    extras.pop(0)[1]()
            for _, e in extras:
                e()
            prev = cur
        # tail: PV + normalize for the last chunk, with WO 16-23 (already
        # normalized) interleaved; then the final WO tiles
        for s in range(16):
            emit_pv_step(prev, s)
            if s % 2 == 1:
                emit_wo_tile(16 + s // 2, use_act=True)
        for tt in range(24, 32):
            emit_wo_tile(tt, use_act=True)


def _prep_in_maps(x, wq, bq, wk, bk, wv, bv, wo):
    x2 = np.asarray(x, np.float32).reshape(NT, D)
    xT = np.ascontiguousarray(x2.T).astype(NPBF16)
    wq = np.asarray(wq, np.float32)
    wk = np.asarray(wk, np.float32)
    wv = np.asarray(wv, np.float32)
    wo = np.asarray(wo, np.float32)
    bq = np.asarray(bq, np.float32)
    bk = np.asarray(bk, np.float32)
    bv = np.asarray(bv, np.float32)
    in_maps = []
    for c in range(NCORES):
        cs = slice(c * 128, (c + 1) * 128)
        in_maps.append({
            "xT": xT,
            "wq": wq[:, cs].astype(NPBF16),
            "wk": wk[:, cs].astype(NPBF16),
            "wv": wv[:, cs].astype(NPBF16),
            "bq": np.ascontiguousarray(bq[cs].reshape(128, 1)),
            "bk": np.ascontiguousarray(bk[cs].reshape(128, 1)),
            "bv": np.ascontiguousarray(bv[cs].reshape(128, 1)),
            "wo": wo[cs, :].astype(NPBF16),
        })
    return in_maps


def kernel(x, wq, bq, wk, bk, wv, bv, wo, bo, _run_kwargs=None):
    if "nc" not in _CACHE:
        _CACHE["nc"] = _build_nc()
    nc = _CACHE["nc"]
    in_maps = _prep_in_maps(x, wq, bq, wk, bk, wv, bv, wo)
    res = run_bass_kernel_spmd(nc, in_maps, list(range(NCORES)),
                               **(_run_kwargs or {}))
    acc = np.zeros((NT, D), np.float32)
    for c in range(NCORES):
        acc += res.results[c]["out"].astype(np.float32)
    acc += np.asarray(bo, np.float32)[None, :]
    if _run_kwargs:
        _CACHE["last_results"] = res
    return acc.reshape(B, S, D)
